# revision 25
# baseline (speedup 1.0000x reference)
"""Trainium2 Bass kernel for multi-head causal attention with RoPE.

Problem: B=4, T=2048, D=2048, H=16 heads (HD=128), fp32 reference:
  q/k/v = x @ w{q,k,v}.T ; RoPE(q,k) ; causal softmax(q k^T/sqrt(HD)) @ v ; @ wo.T

Sharding over 8 cores: 4 batch shards x 2 head-groups (8 heads each).
Each core: projections for its heads (column-split weights), attention,
per-head AllGather of attention outputs within the batch pair (overlapped
with attention compute), then output-column-split wo matmul with
chunk-progressive PSUM accumulation so 7/8 of the wo work runs while the
last head's gather is still in flight.

All matmuls run in fp16 (PE at 1 cycle/row, better mantissa than bf16);
accumulation + softmax in fp32. Attention computes S^T = k^T q per block
so no probs transpose is needed. Softmax denominators are accumulated
elementwise over key-blocks on the Vector engine (fp16) and reduced
across partitions with a single ones-matmul per (head, q-block) instead
of one per key-block, which removes ~10% of all PE work. Exponentials
are batched two key-blocks at a time ([128,1024] activations) to halve
the Activation-engine instruction overhead. Diagonal (causally partial)
blocks use dedicated ptu tiles whose masked prefix is zeroed once and
never rewritten, so no per-block memsets are needed and the GpSimd queue
stays free for the collectives.
"""

import math
import sys
from contextlib import ExitStack

sys.path.insert(0, "/opt/trn_rl_repo")

import numpy as np

import concourse.bass as bass
import concourse.mybir as mybir
import concourse.tile as tile
from concourse import bacc
from concourse.bass_utils import run_bass_kernel_spmd

F16 = np.float16
B, T, D, H, HD = 4, 2048, 2048, 16, 128
HL = 8            # heads per core
DL = HL * HD      # local feature width (1024)
P = 128
NB = 512          # free-dim block for matmuls
N_CORES = 8
INV_SQRT_HD = 1.0 / math.sqrt(HD)

dt = mybir.dt
f32 = dt.float32
f16 = dt.float16


def build_program(t=T, n_cores=N_CORES):
    """Build + compile the per-core Bass program (SPMD, identical on all cores)."""
    n_tb = t // NB      # 512-wide token blocks
    n_tt = t // P       # 128-wide token tiles
    n_db = D // P       # contraction blocks over model dim
    n_q = t // NB       # query blocks (512)

    mult = mybir.AluOpType.mult
    addop = mybir.AluOpType.add
    Exp = mybir.ActivationFunctionType.Exp

    nc = bacc.Bacc("TRN2", target_bir_lowering=False, debug=False,
                   num_devices=n_cores)

    xT = nc.dram_tensor("xT", [D, t], f16, kind="ExternalInput").ap()
    wqT = nc.dram_tensor("wqT", [D, DL], f16, kind="ExternalInput").ap()
    wkT = nc.dram_tensor("wkT", [D, DL], f16, kind="ExternalInput").ap()
    wvT = nc.dram_tensor("wvT", [D, DL], f16, kind="ExternalInput").ap()
    woT = nc.dram_tensor("woT", [D, DL], f16, kind="ExternalInput").ap()
    cosh = nc.dram_tensor("cosh", [P, t], f16, kind="ExternalInput").ap()
    sinh = nc.dram_tensor("sinh", [P, t], f16, kind="ExternalInput").ap()
    trimulT = nc.dram_tensor("trimulT", [P, P], f16, kind="ExternalInput").ap()
    out_part = nc.dram_tensor("out_part", [t, DL], f32, kind="ExternalOutput").ap()

    with tile.TileContext(nc) as tc:
        with tc.tile_pool(name="dram", bufs=1, space="DRAM") as dram:
            # one gather chunk per head: local [128, t], gathered [256, t]
            attnLc = [dram.tile([P, t], f16, name=f"attnL{c}")
                      for c in range(HL)]
            attnFc = [dram.tile([2 * P, t], f16, name=f"attnF{c}")
                      for c in range(HL)]

            with ExitStack() as es:
                persist = es.enter_context(tc.tile_pool(name="persist", bufs=1))
                wvpool_cm = tc.tile_pool(name="wvpool", bufs=1)
                wvpool = wvpool_cm.__enter__()
                xpool_cm = tc.tile_pool(name="xpool", bufs=2)
                xpool = xpool_cm.__enter__()
                qT = [persist.tile([P, t], f16, tag=f"qT{h}", name=f"qT{h}")
                      for h in range(HL)]
                kT = [persist.tile([P, t], f16, tag=f"kT{h}", name=f"kT{h}")
                      for h in range(HL)]
                tri_sb = persist.tile([P, P], f16, tag="tri")
                ones_sb = persist.tile([P, P], f16, tag="ones")
                nc.vector.memset(ones_sb[:], 1.0)
                wv_t = [wvpool.tile([P, DL], f16, tag=f"wv{db}",
                                    name=f"wv{db}") for db in range(n_db)]

                # ============ Phase 1a: Q+K projections (share x tiles) ======
                with (
                    tc.tile_pool(name="ropec", bufs=1) as ropec,
                    tc.tile_pool(name="wpool", bufs=1) as wpool,
                    tc.tile_pool(name="pj_psum", bufs=6, space="PSUM") as pjp,
                    tc.tile_pool(name="rope_tmp", bufs=2) as rtmp,
                ):
                    wq_t = [wpool.tile([P, DL], f16, tag=f"wq{db}",
                                       name=f"wq{db}") for db in range(n_db)]
                    wk_t = [wpool.tile([P, DL], f16, tag=f"wk{db}",
                                       name=f"wk{db}") for db in range(n_db)]
                    cos_sb = ropec.tile([P, t], f16, tag="cos")
                    sin_sb = ropec.tile([P, t], f16, tag="sin")

                    def rope_evict(ps, dst_tile, tsl):
                        u = rtmp.tile([P, NB], f16, tag="u", name="u")
                        nc.vector.tensor_tensor(u[0:64, :], ps[64:128, :],
                                                sin_sb[0:64, tsl], op=mult)
                        nc.vector.tensor_tensor(u[64:128, :], ps[0:64, :],
                                                sin_sb[64:128, tsl], op=mult)
                        nc.vector.tensor_tensor(dst_tile, ps[:], cos_sb[:, tsl],
                                                op=mult)
                        nc.vector.tensor_tensor(dst_tile, dst_tile, u[:], op=addop)

                    for tb in range(n_tb):
                        tsl = bass.ts(tb, NB)
                        x_t = [xpool.tile([P, NB], f16, tag=f"x{db}",
                                          name=f"x{db}") for db in range(n_db)]
                        for db in range(n_db):
                            nc.sync.dma_start(x_t[db][:],
                                              xT[db * P:(db + 1) * P, tsl])
                            if tb == 0:
                                # interleave weight loads with x so the first
                                # matmuls unblock as early as possible
                                nc.sync.dma_start(wq_t[db][:],
                                                  wqT[db * P:(db + 1) * P, :])
                        if tb == 0:
                            # emit late-needed loads behind the critical path
                            nc.sync.dma_start(cos_sb[:], cosh[:])
                            nc.sync.dma_start(sin_sb[:], sinh[:])
                            nc.sync.dma_start(tri_sb[:], trimulT[:])
                            for db in range(n_db):
                                nc.sync.dma_start(wk_t[db][:],
                                                  wkT[db * P:(db + 1) * P, :])
                        if tb == 1:
                            for db in range(n_db):
                                nc.sync.dma_start(wv_t[db][:],
                                                  wvT[db * P:(db + 1) * P, :])
                        # all-Q then all-K: K weights arrive after Q's, so
                        # keep the PE busy on Q while wk streams in
                        for (w_t, dst) in ((wq_t, qT), (wk_t, kT)):
                            for jt in range(HL):
                                ps = pjp.tile([P, NB], f32, name="ps")
                                for db in range(n_db):
                                    nc.tensor.matmul(
                                        ps[:], lhsT=w_t[db][:, bass.ts(jt, P)],
                                        rhs=x_t[db][:],
                                        start=(db == 0), stop=(db == n_db - 1))
                                rope_evict(ps, dst[jt][:, tsl], tsl)

                # ============ Phase 1b: V projection =========================
                vpool = es.enter_context(
                    tc.tile_pool(name="vpool", bufs=1, side="right"))
                v = [vpool.tile([P, DL], f16, tag=f"v{tt}", name=f"v{tt}")
                     for tt in range(n_tt)]
                with tc.tile_pool(name="v_psum", bufs=3, space="PSUM") as vps:
                    for tb in range(n_tb):
                        x_t = [xpool.tile([P, NB], f16, tag=f"x{db}",
                                          name=f"x{db}") for db in range(n_db)]
                        for db in range(n_db):
                            nc.sync.dma_start(
                                x_t[db][:], xT[db * P:(db + 1) * P, bass.ts(tb, NB)])
                        for tq in range(4):
                            tt = tb * 4 + tq
                            # jb innermost: consecutive matmuls share lhsT so
                            # the PE weight load amortizes over two streams
                            pp = [vps.tile([P, NB], f32, tag=f"vp{jb}",
                                           name=f"vp{jb}")
                                  for jb in range(DL // NB)]
                            for db in range(n_db):
                                for jb in range(DL // NB):
                                    nc.tensor.matmul(
                                        pp[jb][:], lhsT=x_t[db][:, bass.ts(tq, P)],
                                        rhs=wv_t[db][:, bass.ts(jb, NB)],
                                        start=(db == 0), stop=(db == n_db - 1))
                            for jb in range(DL // NB):
                                nc.vector.tensor_copy(v[tt][:, bass.ts(jb, NB)],
                                                      pp[jb][:])
                xpool_cm.__exit__(None, None, None)
                wvpool_cm.__exit__(None, None, None)

                # wo weights prefetch: loads overlap the attention phase
                wopool = es.enter_context(
                    tc.tile_pool(name="wopool", bufs=1, side="right"))
                n_fb = (2 * DL) // P
                wo_t = [wopool.tile([P, DL], f16, tag=f"wo{fb}", name=f"wo{fb}")
                        for fb in range(n_fb)]
                for fb in range(n_fb):
                    nc.sync.dma_start(wo_t[fb][:], woT[fb * P:(fb + 1) * P, :])
                # af tiles for the first two wo token-groups are DMA'd inside
                # the attention loop (right after each head's gather) so the
                # wo phase starts with its inputs already resident
                afp = es.enter_context(tc.tile_pool(name="afpool", bufs=2))
                af_pre = [[afp.tile([P, 4 * P], f16, tag=f"af{fb}",
                                    name=f"af{fb}") for fb in range(n_fb)]
                          for _ in range(2)]

                # ============ Phase 2: attention (+ per-head gather) ==========
                with (
                    tc.tile_pool(name="st_psum", bufs=2, space="PSUM") as stp,
                    tc.tile_pool(name="ot_psum", bufs=2, space="PSUM") as otp,
                    tc.tile_pool(name="sum_psum", bufs=2, space="PSUM") as smp,
                    tc.tile_pool(name="ptuf", bufs=3) as ptufp,
                    tc.tile_pool(name="accp", bufs=2) as accp,
                    tc.tile_pool(name="att_ev", bufs=4) as atev,
                    tc.tile_pool(name="ptud", bufs=1) as ptudp,
                ):
                    # dedicated diagonal-block tiles: prefix cols (future keys
                    # at 128-granularity) are zeroed once and never rewritten
                    ptu_r = [[ptudp.tile([P, NB], f16, tag=f"ptud{rel}_{par}",
                                         name=f"ptud{rel}_{par}")
                              for par in range(2)] for rel in range(4)]
                    for rel in range(1, 4):
                        for par in range(2):
                            nc.vector.memset(ptu_r[rel][par][:, 0:rel * P], 0.0)

                    for h in range(HL):
                        for q in range(n_q):
                            qsl = bass.ts(q, NB)
                            nkb = 4 * q + 4
                            nfull = 4 * q
                            par = (h * n_q + q) % 2
                            ot = otp.tile([P, NB], f32, tag="ot", name="ot")
                            acc = accp.tile([P, NB], f16, tag="acc", name="acc")
                            acc_started = False
                            # ---- full key-blocks, two per [128,1024] psum ----
                            for pi in range(nfull // 2):
                                st = stp.tile([P, 2 * NB], f32, tag="st",
                                              name="st")
                                for hf in range(2):
                                    kb = 2 * pi + hf
                                    nc.tensor.matmul(
                                        st[:, bass.ts(hf, NB)],
                                        lhsT=kT[h][:, bass.ts(kb, P)],
                                        rhs=qT[h][:, qsl], start=True, stop=True)
                                ptu = ptufp.tile([P, 2 * NB], f16, tag="ptuf",
                                                 name="ptuf")
                                nc.scalar.activation(ptu[:], st[:], Exp,
                                                     scale=INV_SQRT_HD)
                                for hf in range(2):
                                    kb = 2 * pi + hf
                                    nc.tensor.matmul(
                                        ot[:], lhsT=v[kb][:, bass.ts(h, P)],
                                        rhs=ptu[:, bass.ts(hf, NB)],
                                        start=(kb == 0), stop=False)
                                if not acc_started:
                                    nc.vector.tensor_tensor(
                                        acc[:], ptu[:, 0:NB], ptu[:, NB:2 * NB],
                                        op=addop)
                                    acc_started = True
                                else:
                                    nc.vector.tensor_tensor(
                                        acc[:], acc[:], ptu[:, 0:NB], op=addop)
                                    nc.vector.tensor_tensor(
                                        acc[:], acc[:], ptu[:, NB:2 * NB],
                                        op=addop)
                            # ---- diagonal key-blocks (rel = kb - 4q in 0..3) --
                            diag_pt = []
                            for pi in range(2):
                                st = stp.tile([P, 2 * NB], f32, tag="st",
                                              name="st")
                                for hf in range(2):
                                    rel = 2 * pi + hf
                                    kb = nfull + rel
                                    nc.tensor.matmul(
                                        st[:, bass.ts(hf, NB)],
                                        lhsT=kT[h][:, bass.ts(kb, P)],
                                        rhs=qT[h][:, qsl], start=True, stop=True)
                                for hf in range(2):
                                    rel = 2 * pi + hf
                                    kb = nfull + rel
                                    pt = ptu_r[rel][par]
                                    ew = NB - rel * P
                                    nc.scalar.activation(
                                        pt[:, bass.ds(rel * P, ew)],
                                        st[:, bass.ds(hf * NB + rel * P, ew)],
                                        Exp, scale=INV_SQRT_HD)
                                    nc.vector.tensor_tensor(
                                        pt[:, bass.ds(rel * P, P)],
                                        pt[:, bass.ds(rel * P, P)], tri_sb[:],
                                        op=mult)
                                    nc.tensor.matmul(
                                        ot[:], lhsT=v[kb][:, bass.ts(h, P)],
                                        rhs=pt[:], start=(kb == 0),
                                        stop=(kb == nkb - 1))
                                    diag_pt.append(pt)
                            if not acc_started:
                                nc.vector.tensor_tensor(
                                    acc[:], diag_pt[0][:], diag_pt[1][:],
                                    op=addop)
                                for pt in diag_pt[2:]:
                                    nc.vector.tensor_tensor(acc[:], acc[:],
                                                            pt[:], op=addop)
                            else:
                                for pt in diag_pt:
                                    nc.vector.tensor_tensor(acc[:], acc[:],
                                                            pt[:], op=addop)
                            # ---- normalize: one partition-reduce matmul ------
                            sums = smp.tile([P, NB], f32, tag="sums",
                                            name="sums")
                            nc.tensor.matmul(sums[:], lhsT=ones_sb[:],
                                             rhs=acc[:], start=True, stop=True)
                            rb = atev.tile([P, NB], f32, tag="rb", name="rb")
                            nc.vector.reciprocal_approx_fast(out=rb[:],
                                                             in_=sums[:])
                            att = atev.tile([P, NB], f16, tag="att", name="att")
                            nc.vector.tensor_tensor(att[:], ot[:], rb[:],
                                                    op=mult)
                            nc.sync.dma_start(attnLc[h][:, qsl], att[:])
                        # per-head gather, overlapped with remaining heads
                        nc.gpsimd.collective_compute(
                            "AllGather", mybir.AluOpType.bypass,
                            replica_groups=[[i, i + 1]
                                            for i in range(0, n_cores, 2)],
                            ins=[attnLc[h].opt()], outs=[attnFc[h].opt()],
                        )
                        for g in range(2):
                            for r in range(2):
                                fb = 2 * h + r
                                nc.sync.dma_start(
                                    af_pre[g][fb][:],
                                    attnFc[h][r * P:(r + 1) * P,
                                              bass.ts(g, 4 * P)])

                # ============ Phase 3: wo matmul (chunk-progressive) ========
                # fb = 2*c + r: chunk c rows [own head c | partner head c]
                with (
                    tc.tile_pool(name="wo_psum", bufs=1, space="PSUM") as wps,
                    tc.tile_pool(name="out_ev", bufs=4) as oev,
                ):
                    af_next = None
                    for grp in range(n_tt // 4):
                        af_g = af_pre[grp] if grp < 2 else af_next
                        if 2 <= grp + 1 < n_tt // 4:
                            # issue the next group's input loads now: they only
                            # wait on a retired generation's readers, so the
                            # transfer overlaps this group's matmuls
                            ngsl = bass.ts(grp + 1, 4 * P)
                            af_next = [afp.tile([P, 4 * P], f16, tag=f"af{fb}",
                                                name=f"af{fb}")
                                       for fb in range(n_fb)]
                            for c in range(HL):
                                for r in range(2):
                                    fb = 2 * c + r
                                    nc.sync.dma_start(
                                        af_next[fb][:],
                                        attnFc[c][r * P:(r + 1) * P, ngsl])
                        pss = [wps.tile([P, NB], f32, tag=f"ps{i}",
                                        name=f"ps{i}") for i in range(8)]
                        for c in range(HL):
                            for ti in range(4):
                                for r in range(2):
                                    fb = 2 * c + r
                                    # ob innermost: the two streams share lhsT
                                    for ob in range(DL // NB):
                                        nc.tensor.matmul(
                                            pss[ti * 2 + ob][:],
                                            lhsT=af_g[fb][:, bass.ts(ti, P)],
                                            rhs=wo_t[fb][:, bass.ts(ob, NB)],
                                            start=(c == 0 and r == 0),
                                            stop=(c == HL - 1 and r == 1))
                                if c == HL - 1:
                                    # evict as soon as each token-tile stops so
                                    # the drain overlaps the remaining matmuls
                                    tt = grp * 4 + ti
                                    for ob in range(DL // NB):
                                        o = oev.tile([P, NB], f32, tag="o",
                                                     name="o")
                                        nc.scalar.copy(o[:], pss[ti * 2 + ob][:])
                                        nc.sync.dma_start(
                                            out_part[tt * P:(tt + 1) * P,
                                                     bass.ts(ob, NB)], o[:])

    nc.compile()
    return nc


# ---------------- host side ----------------

_ROPE_PERM = np.concatenate([np.arange(0, HD, 2), np.arange(1, HD, 2)])


def host_prep(inputs, t=T):
    """Build per-core input maps from the full problem inputs."""
    x = np.asarray(inputs["x"])[:, :t, :]
    wq, wk, wv, wo = (np.asarray(inputs[k]) for k in ("wq", "wk", "wv", "wo"))
    fcos = np.asarray(inputs["freqs_cos"])[:t]
    fsin = np.asarray(inputs["freqs_sin"])[:t]
    mask = np.asarray(inputs["mask"])

    cosT = np.ascontiguousarray(fcos.T)          # (64, t)
    sinT = np.ascontiguousarray(fsin.T)
    cosh = np.concatenate([cosT, cosT], 0).astype(F16)    # (128, t)
    sinh = np.concatenate([-sinT, sinT], 0).astype(F16)
    # multiplicative mask tile: exp(mask) on the transposed diagonal block
    # (reference computes softmax(s/sqrt(HD) + mask), and exp(a+m)=exp(a)exp(m))
    with np.errstate(over="ignore"):
        trimulT = np.exp(np.ascontiguousarray(mask[0:P, 0:P].T)).astype(F16)

    perm = np.concatenate([h * HD + _ROPE_PERM for h in range(HL)])

    # wo input-feature order after per-head gather:
    # chunk c = [g0 head c | g1 head c]
    forder = np.empty(2 * DL, np.int64)
    for c in range(HL):
        base = c * 2 * HD
        forder[base:base + HD] = np.arange(c * HD, (c + 1) * HD)
        forder[base + HD:base + 2 * HD] = DL + np.arange(c * HD, (c + 1) * HD)

    xTs = [np.ascontiguousarray(x[b].astype(F16).T) for b in range(B)]
    per_g = []
    for g in range(2):
        sl = slice(g * DL, (g + 1) * DL)
        per_g.append({
            "wqT": np.ascontiguousarray(wq[sl][perm].astype(F16).T),
            "wkT": np.ascontiguousarray(wk[sl][perm].astype(F16).T),
            "wvT": np.ascontiguousarray(wv[sl].astype(F16).T),
            "woT": np.ascontiguousarray(wo[sl][:, forder].astype(F16).T),
        })

    in_maps = []
    for c in range(N_CORES):
        b, g = c // 2, c % 2
        m = {"xT": xTs[b], "cosh": cosh, "sinh": sinh, "trimulT": trimulT}
        m.update(per_g[g])
        in_maps.append(m)
    return in_maps


_PROGRAM_CACHE = {}


def get_program(t=T, n_cores=N_CORES):
    key = (t, n_cores)
    if key not in _PROGRAM_CACHE:
        _PROGRAM_CACHE[key] = build_program(t, n_cores)
    return _PROGRAM_CACHE[key]


def assemble(results, t=T):
    out = np.empty((B, t, D), np.float32)
    for c in range(N_CORES):
        b, g = c // 2, c % 2
        out[b, :, g * DL:(g + 1) * DL] = results[c]["out_part"]
    return out


def kernel(**inputs):
    nc = get_program()
    in_maps = host_prep(inputs)
    res = run_bass_kernel_spmd(nc, in_maps, core_ids=list(range(N_CORES)))
    return assemble(res.results)


# revision 26
# speedup vs baseline: 1.0181x; 1.0181x over previous
"""Trainium2 Bass kernel for multi-head causal attention with RoPE.

Problem: B=4, T=2048, D=2048, H=16 heads (HD=128), fp32 reference:
  q/k/v = x @ w{q,k,v}.T ; RoPE(q,k) ; causal softmax(q k^T/sqrt(HD)) @ v ; @ wo.T

Sharding over 8 cores: 4 batch shards x 2 head-groups (8 heads each).
Each core: projections for its heads (column-split weights), attention,
per-head AllGather of attention outputs within the batch pair (overlapped
with attention compute), then output-column-split wo matmul with
chunk-progressive PSUM accumulation so 7/8 of the wo work runs while the
last head's gather is still in flight.

All matmuls run in fp16 (PE at 1 cycle/row, better mantissa than bf16);
accumulation + softmax in fp32. Attention computes S^T = k^T q per block
so no probs transpose is needed. Softmax denominators are accumulated
elementwise over key-blocks on the Vector engine (fp16) and reduced
across partitions with a single ones-matmul per (head, q-block) instead
of one per key-block, which removes ~10% of all PE work. Exponentials
are batched two key-blocks at a time ([128,1024] activations) to halve
the Activation-engine instruction overhead. Diagonal (causally partial)
blocks use dedicated ptu tiles whose masked prefix is zeroed once and
never rewritten, so no per-block memsets are needed and the GpSimd queue
stays free for the collectives.
"""

import math
import sys
from contextlib import ExitStack

sys.path.insert(0, "/opt/trn_rl_repo")

import numpy as np

import concourse.bass as bass
import concourse.mybir as mybir
import concourse.tile as tile
from concourse import bacc
from concourse.bass_utils import run_bass_kernel_spmd

F16 = np.float16
B, T, D, H, HD = 4, 2048, 2048, 16, 128
HL = 8            # heads per core
DL = HL * HD      # local feature width (1024)
P = 128
NB = 512          # free-dim block for matmuls
N_CORES = 8
INV_SQRT_HD = 1.0 / math.sqrt(HD)

dt = mybir.dt
f32 = dt.float32
f16 = dt.float16


def build_program(t=T, n_cores=N_CORES):
    """Build + compile the per-core Bass program (SPMD, identical on all cores)."""
    n_tb = t // NB      # 512-wide token blocks
    n_tt = t // P       # 128-wide token tiles
    n_db = D // P       # contraction blocks over model dim
    n_q = t // NB       # query blocks (512)

    mult = mybir.AluOpType.mult
    addop = mybir.AluOpType.add
    Exp = mybir.ActivationFunctionType.Exp

    nc = bacc.Bacc("TRN2", target_bir_lowering=False, debug=False,
                   num_devices=n_cores)

    xT = nc.dram_tensor("xT", [D, t], f16, kind="ExternalInput").ap()
    wqT = nc.dram_tensor("wqT", [D, DL], f16, kind="ExternalInput").ap()
    wkT = nc.dram_tensor("wkT", [D, DL], f16, kind="ExternalInput").ap()
    wvT = nc.dram_tensor("wvT", [D, DL], f16, kind="ExternalInput").ap()
    woT = nc.dram_tensor("woT", [D, DL], f16, kind="ExternalInput").ap()
    cosh = nc.dram_tensor("cosh", [P, t], f16, kind="ExternalInput").ap()
    sinh = nc.dram_tensor("sinh", [P, t], f16, kind="ExternalInput").ap()
    trimulT = nc.dram_tensor("trimulT", [P, P], f16, kind="ExternalInput").ap()
    out_part = nc.dram_tensor("out_part", [t, DL], f32, kind="ExternalOutput").ap()

    with tile.TileContext(nc) as tc:
        with tc.tile_pool(name="dram", bufs=1, space="DRAM") as dram:
            # one gather chunk per head: local [128, t], gathered [256, t]
            attnLc = [dram.tile([P, t], f16, name=f"attnL{c}")
                      for c in range(HL)]
            attnFc = [dram.tile([2 * P, t], f16, name=f"attnF{c}")
                      for c in range(HL)]

            with ExitStack() as es:
                persist = es.enter_context(tc.tile_pool(name="persist", bufs=1))
                wvpool_cm = tc.tile_pool(name="wvpool", bufs=1)
                wvpool = wvpool_cm.__enter__()
                xpool_cm = tc.tile_pool(name="xpool", bufs=2)
                xpool = xpool_cm.__enter__()
                qT = [persist.tile([P, t], f16, tag=f"qT{h}", name=f"qT{h}")
                      for h in range(HL)]
                kT = [persist.tile([P, t], f16, tag=f"kT{h}", name=f"kT{h}")
                      for h in range(HL)]
                tri_sb = persist.tile([P, P], f16, tag="tri")
                ones_sb = persist.tile([P, P], f16, tag="ones")
                nc.vector.memset(ones_sb[:], 1.0)
                wv_t = [wvpool.tile([P, DL], f16, tag=f"wv{db}",
                                    name=f"wv{db}") for db in range(n_db)]

                # ============ Phase 1a: Q+K projections (share x tiles) ======
                with (
                    tc.tile_pool(name="ropec", bufs=1) as ropec,
                    tc.tile_pool(name="wpool", bufs=1) as wpool,
                    tc.tile_pool(name="pj_psum", bufs=8, space="PSUM") as pjp,
                    tc.tile_pool(name="rope_tmp", bufs=2) as rtmp,
                ):
                    wq_t = [wpool.tile([P, DL], f16, tag=f"wq{db}",
                                       name=f"wq{db}") for db in range(n_db)]
                    wk_t = [wpool.tile([P, DL], f16, tag=f"wk{db}",
                                       name=f"wk{db}") for db in range(n_db)]
                    cos_sb = ropec.tile([P, t], f16, tag="cos")
                    sin_sb = ropec.tile([P, t], f16, tag="sin")

                    def rope_evict(ps, dst_tile, tsl):
                        u = rtmp.tile([P, NB], f16, tag="u", name="u")
                        nc.vector.tensor_tensor(u[0:64, :], ps[64:128, :],
                                                sin_sb[0:64, tsl], op=mult)
                        nc.vector.tensor_tensor(u[64:128, :], ps[0:64, :],
                                                sin_sb[64:128, tsl], op=mult)
                        nc.vector.tensor_tensor(dst_tile, ps[:], cos_sb[:, tsl],
                                                op=mult)
                        nc.vector.tensor_tensor(dst_tile, dst_tile, u[:], op=addop)

                    for tb in range(n_tb):
                        tsl = bass.ts(tb, NB)
                        x_t = [xpool.tile([P, NB], f16, tag=f"x{db}",
                                          name=f"x{db}") for db in range(n_db)]
                        for db in range(n_db):
                            nc.sync.dma_start(x_t[db][:],
                                              xT[db * P:(db + 1) * P, tsl])
                            if tb == 0:
                                # interleave weight loads with x so the first
                                # matmuls unblock as early as possible
                                nc.sync.dma_start(wq_t[db][:],
                                                  wqT[db * P:(db + 1) * P, :])
                        if tb == 0:
                            # emit late-needed loads behind the critical path
                            nc.sync.dma_start(cos_sb[:], cosh[:])
                            nc.sync.dma_start(sin_sb[:], sinh[:])
                            nc.sync.dma_start(tri_sb[:], trimulT[:])
                            for db in range(n_db):
                                nc.sync.dma_start(wk_t[db][:],
                                                  wkT[db * P:(db + 1) * P, :])
                        if tb == 1:
                            for db in range(n_db):
                                nc.sync.dma_start(wv_t[db][:],
                                                  wvT[db * P:(db + 1) * P, :])
                        # all-Q then all-K: K weights arrive after Q's, so
                        # keep the PE busy on Q while wk streams in
                        for (w_t, dst) in ((wq_t, qT), (wk_t, kT)):
                            for jt in range(HL):
                                ps = pjp.tile([P, NB], f32, name="ps")
                                for db in range(n_db):
                                    nc.tensor.matmul(
                                        ps[:], lhsT=w_t[db][:, bass.ts(jt, P)],
                                        rhs=x_t[db][:],
                                        start=(db == 0), stop=(db == n_db - 1))
                                rope_evict(ps, dst[jt][:, tsl], tsl)

                # ============ Phase 1b: V projection =========================
                vpool = es.enter_context(
                    tc.tile_pool(name="vpool", bufs=1, side="right"))
                v = [vpool.tile([P, DL], f16, tag=f"v{tt}", name=f"v{tt}")
                     for tt in range(n_tt)]
                with tc.tile_pool(name="v_psum", bufs=4, space="PSUM") as vps:
                    for tb in range(n_tb):
                        x_t = [xpool.tile([P, NB], f16, tag=f"x{db}",
                                          name=f"x{db}") for db in range(n_db)]
                        for db in range(n_db):
                            nc.sync.dma_start(
                                x_t[db][:], xT[db * P:(db + 1) * P, bass.ts(tb, NB)])
                        for tq in range(4):
                            tt = tb * 4 + tq
                            # jb innermost: consecutive matmuls share lhsT so
                            # the PE weight load amortizes over two streams
                            pp = [vps.tile([P, NB], f32, tag=f"vp{jb}",
                                           name=f"vp{jb}")
                                  for jb in range(DL // NB)]
                            for db in range(n_db):
                                for jb in range(DL // NB):
                                    nc.tensor.matmul(
                                        pp[jb][:], lhsT=x_t[db][:, bass.ts(tq, P)],
                                        rhs=wv_t[db][:, bass.ts(jb, NB)],
                                        start=(db == 0), stop=(db == n_db - 1))
                            for jb in range(DL // NB):
                                nc.vector.tensor_copy(v[tt][:, bass.ts(jb, NB)],
                                                      pp[jb][:])
                xpool_cm.__exit__(None, None, None)
                wvpool_cm.__exit__(None, None, None)

                # wo weights prefetch: loads overlap the attention phase
                wopool = es.enter_context(
                    tc.tile_pool(name="wopool", bufs=1, side="right"))
                n_fb = (2 * DL) // P
                wo_t = [wopool.tile([P, DL], f16, tag=f"wo{fb}", name=f"wo{fb}")
                        for fb in range(n_fb)]
                for fb in range(n_fb):
                    nc.sync.dma_start(wo_t[fb][:], woT[fb * P:(fb + 1) * P, :])
                # af tiles for the first two wo token-groups are DMA'd inside
                # the attention loop (right after each head's gather) so the
                # wo phase starts with its inputs already resident
                afp = es.enter_context(tc.tile_pool(name="afpool", bufs=2))
                af_pre = [[afp.tile([P, 4 * P], f16, tag=f"af{fb}",
                                    name=f"af{fb}") for fb in range(n_fb)]
                          for _ in range(2)]

                # ============ Phase 2: attention (+ per-head gather) ==========
                with (
                    tc.tile_pool(name="st_psum", bufs=2, space="PSUM") as stp,
                    tc.tile_pool(name="ot_psum", bufs=2, space="PSUM") as otp,
                    tc.tile_pool(name="sum_psum", bufs=2, space="PSUM") as smp,
                    tc.tile_pool(name="ptuf", bufs=3) as ptufp,
                    tc.tile_pool(name="accp", bufs=2) as accp,
                    tc.tile_pool(name="att_ev", bufs=4) as atev,
                    tc.tile_pool(name="ptud", bufs=1) as ptudp,
                ):
                    # dedicated diagonal-block tiles: prefix cols (future keys
                    # at 128-granularity) are zeroed once and never rewritten
                    ptu_r = [[ptudp.tile([P, NB], f16, tag=f"ptud{rel}_{par}",
                                         name=f"ptud{rel}_{par}")
                              for par in range(2)] for rel in range(4)]
                    for rel in range(1, 4):
                        for par in range(2):
                            nc.vector.memset(ptu_r[rel][par][:, 0:rel * P], 0.0)

                    for h in range(HL):
                        for q in range(n_q):
                            qsl = bass.ts(q, NB)
                            nkb = 4 * q + 4
                            nfull = 4 * q
                            par = (h * n_q + q) % 2
                            ot = otp.tile([P, NB], f32, tag="ot", name="ot")
                            acc = accp.tile([P, NB], f16, tag="acc", name="acc")
                            acc_started = False
                            # ---- full key-blocks, two per [128,1024] psum ----
                            for pi in range(nfull // 2):
                                st = stp.tile([P, 2 * NB], f32, tag="st",
                                              name="st")
                                for hf in range(2):
                                    kb = 2 * pi + hf
                                    nc.tensor.matmul(
                                        st[:, bass.ts(hf, NB)],
                                        lhsT=kT[h][:, bass.ts(kb, P)],
                                        rhs=qT[h][:, qsl], start=True, stop=True)
                                ptu = ptufp.tile([P, 2 * NB], f16, tag="ptuf",
                                                 name="ptuf")
                                nc.scalar.activation(ptu[:], st[:], Exp,
                                                     scale=INV_SQRT_HD)
                                for hf in range(2):
                                    kb = 2 * pi + hf
                                    nc.tensor.matmul(
                                        ot[:], lhsT=v[kb][:, bass.ts(h, P)],
                                        rhs=ptu[:, bass.ts(hf, NB)],
                                        start=(kb == 0), stop=False)
                                if not acc_started:
                                    nc.vector.tensor_tensor(
                                        acc[:], ptu[:, 0:NB], ptu[:, NB:2 * NB],
                                        op=addop)
                                    acc_started = True
                                else:
                                    nc.vector.tensor_tensor(
                                        acc[:], acc[:], ptu[:, 0:NB], op=addop)
                                    nc.vector.tensor_tensor(
                                        acc[:], acc[:], ptu[:, NB:2 * NB],
                                        op=addop)
                            # ---- diagonal key-blocks (rel = kb - 4q in 0..3) --
                            diag_pt = []
                            for pi in range(2):
                                st = stp.tile([P, 2 * NB], f32, tag="st",
                                              name="st")
                                for hf in range(2):
                                    rel = 2 * pi + hf
                                    kb = nfull + rel
                                    nc.tensor.matmul(
                                        st[:, bass.ts(hf, NB)],
                                        lhsT=kT[h][:, bass.ts(kb, P)],
                                        rhs=qT[h][:, qsl], start=True, stop=True)
                                for hf in range(2):
                                    rel = 2 * pi + hf
                                    kb = nfull + rel
                                    pt = ptu_r[rel][par]
                                    ew = NB - rel * P
                                    nc.scalar.activation(
                                        pt[:, bass.ds(rel * P, ew)],
                                        st[:, bass.ds(hf * NB + rel * P, ew)],
                                        Exp, scale=INV_SQRT_HD)
                                    nc.vector.tensor_tensor(
                                        pt[:, bass.ds(rel * P, P)],
                                        pt[:, bass.ds(rel * P, P)], tri_sb[:],
                                        op=mult)
                                    nc.tensor.matmul(
                                        ot[:], lhsT=v[kb][:, bass.ts(h, P)],
                                        rhs=pt[:], start=(kb == 0),
                                        stop=(kb == nkb - 1))
                                    diag_pt.append(pt)
                            if not acc_started:
                                nc.vector.tensor_tensor(
                                    acc[:], diag_pt[0][:], diag_pt[1][:],
                                    op=addop)
                                for pt in diag_pt[2:]:
                                    nc.vector.tensor_tensor(acc[:], acc[:],
                                                            pt[:], op=addop)
                            else:
                                for pt in diag_pt:
                                    nc.vector.tensor_tensor(acc[:], acc[:],
                                                            pt[:], op=addop)
                            # ---- normalize: one partition-reduce matmul ------
                            sums = smp.tile([P, NB], f32, tag="sums",
                                            name="sums")
                            nc.tensor.matmul(sums[:], lhsT=ones_sb[:],
                                             rhs=acc[:], start=True, stop=True)
                            rb = atev.tile([P, NB], f32, tag="rb", name="rb")
                            nc.vector.reciprocal_approx_fast(out=rb[:],
                                                             in_=sums[:])
                            att = atev.tile([P, NB], f16, tag="att", name="att")
                            nc.vector.tensor_tensor(att[:], ot[:], rb[:],
                                                    op=mult)
                            nc.sync.dma_start(attnLc[h][:, qsl], att[:])
                        # per-head gather, overlapped with remaining heads
                        nc.gpsimd.collective_compute(
                            "AllGather", mybir.AluOpType.bypass,
                            replica_groups=[[i, i + 1]
                                            for i in range(0, n_cores, 2)],
                            ins=[attnLc[h].opt()], outs=[attnFc[h].opt()],
                        )
                        for g in range(2):
                            for r in range(2):
                                fb = 2 * h + r
                                nc.sync.dma_start(
                                    af_pre[g][fb][:],
                                    attnFc[h][r * P:(r + 1) * P,
                                              bass.ts(g, 4 * P)])

                # ============ Phase 3: wo matmul (chunk-progressive) ========
                # fb = 2*c + r: chunk c rows [own head c | partner head c]
                with (
                    tc.tile_pool(name="wo_psum", bufs=1, space="PSUM") as wps,
                    tc.tile_pool(name="out_ev", bufs=4) as oev,
                ):
                    af_next = None
                    for grp in range(n_tt // 4):
                        af_g = af_pre[grp] if grp < 2 else af_next
                        if 2 <= grp + 1 < n_tt // 4:
                            # issue the next group's input loads now: they only
                            # wait on a retired generation's readers, so the
                            # transfer overlaps this group's matmuls
                            ngsl = bass.ts(grp + 1, 4 * P)
                            af_next = [afp.tile([P, 4 * P], f16, tag=f"af{fb}",
                                                name=f"af{fb}")
                                       for fb in range(n_fb)]
                            for c in range(HL):
                                for r in range(2):
                                    fb = 2 * c + r
                                    nc.sync.dma_start(
                                        af_next[fb][:],
                                        attnFc[c][r * P:(r + 1) * P, ngsl])
                        pss = [wps.tile([P, NB], f32, tag=f"ps{i}",
                                        name=f"ps{i}") for i in range(8)]
                        for c in range(HL):
                            for ti in range(4):
                                for r in range(2):
                                    fb = 2 * c + r
                                    # ob innermost: the two streams share lhsT
                                    for ob in range(DL // NB):
                                        nc.tensor.matmul(
                                            pss[ti * 2 + ob][:],
                                            lhsT=af_g[fb][:, bass.ts(ti, P)],
                                            rhs=wo_t[fb][:, bass.ts(ob, NB)],
                                            start=(c == 0 and r == 0),
                                            stop=(c == HL - 1 and r == 1))
                                if c == HL - 1:
                                    # evict as soon as each token-tile stops so
                                    # the drain overlaps the remaining matmuls
                                    tt = grp * 4 + ti
                                    for ob in range(DL // NB):
                                        o = oev.tile([P, NB], f32, tag="o",
                                                     name="o")
                                        nc.scalar.copy(o[:], pss[ti * 2 + ob][:])
                                        nc.sync.dma_start(
                                            out_part[tt * P:(tt + 1) * P,
                                                     bass.ts(ob, NB)], o[:])

    nc.compile()
    return nc


# ---------------- host side ----------------

_ROPE_PERM = np.concatenate([np.arange(0, HD, 2), np.arange(1, HD, 2)])


def host_prep(inputs, t=T):
    """Build per-core input maps from the full problem inputs."""
    x = np.asarray(inputs["x"])[:, :t, :]
    wq, wk, wv, wo = (np.asarray(inputs[k]) for k in ("wq", "wk", "wv", "wo"))
    fcos = np.asarray(inputs["freqs_cos"])[:t]
    fsin = np.asarray(inputs["freqs_sin"])[:t]
    mask = np.asarray(inputs["mask"])

    cosT = np.ascontiguousarray(fcos.T)          # (64, t)
    sinT = np.ascontiguousarray(fsin.T)
    cosh = np.concatenate([cosT, cosT], 0).astype(F16)    # (128, t)
    sinh = np.concatenate([-sinT, sinT], 0).astype(F16)
    # multiplicative mask tile: exp(mask) on the transposed diagonal block
    # (reference computes softmax(s/sqrt(HD) + mask), and exp(a+m)=exp(a)exp(m))
    with np.errstate(over="ignore"):
        trimulT = np.exp(np.ascontiguousarray(mask[0:P, 0:P].T)).astype(F16)

    perm = np.concatenate([h * HD + _ROPE_PERM for h in range(HL)])

    # wo input-feature order after per-head gather:
    # chunk c = [g0 head c | g1 head c]
    forder = np.empty(2 * DL, np.int64)
    for c in range(HL):
        base = c * 2 * HD
        forder[base:base + HD] = np.arange(c * HD, (c + 1) * HD)
        forder[base + HD:base + 2 * HD] = DL + np.arange(c * HD, (c + 1) * HD)

    xTs = [np.ascontiguousarray(x[b].astype(F16).T) for b in range(B)]
    per_g = []
    for g in range(2):
        sl = slice(g * DL, (g + 1) * DL)
        per_g.append({
            "wqT": np.ascontiguousarray(wq[sl][perm].astype(F16).T),
            "wkT": np.ascontiguousarray(wk[sl][perm].astype(F16).T),
            "wvT": np.ascontiguousarray(wv[sl].astype(F16).T),
            "woT": np.ascontiguousarray(wo[sl][:, forder].astype(F16).T),
        })

    in_maps = []
    for c in range(N_CORES):
        b, g = c // 2, c % 2
        m = {"xT": xTs[b], "cosh": cosh, "sinh": sinh, "trimulT": trimulT}
        m.update(per_g[g])
        in_maps.append(m)
    return in_maps


_PROGRAM_CACHE = {}


def get_program(t=T, n_cores=N_CORES):
    key = (t, n_cores)
    if key not in _PROGRAM_CACHE:
        _PROGRAM_CACHE[key] = build_program(t, n_cores)
    return _PROGRAM_CACHE[key]


def assemble(results, t=T):
    out = np.empty((B, t, D), np.float32)
    for c in range(N_CORES):
        b, g = c // 2, c % 2
        out[b, :, g * DL:(g + 1) * DL] = results[c]["out_part"]
    return out


def kernel(**inputs):
    nc = get_program()
    in_maps = host_prep(inputs)
    res = run_bass_kernel_spmd(nc, in_maps, core_ids=list(range(N_CORES)))
    return assemble(res.results)
